# revision 1
# baseline (speedup 1.0000x reference)
"""AFNONet (B=16, 160x160, depth 12) forward as a Bass/Tile kernel on 8
TRN2 NeuronCores. Data-parallel: 2 images per core, no collectives.

Per-core kernel: feature-major activations [chan(part), img*pos(free)],
f32r residual stream, LN stats via ones-matmul broadcast trick, rfft2/irfft2
as dense DFT matmuls (bf16), AFNO block-diag mixing as 96x96 bf16 matmuls,
host-pre-transposed bf16 MLP weights, per-image software pipelining so one
image's AFNO elementwise work overlaps the other image's MLP matmuls.
"""
import sys
for _p in ("/opt/trn_rl_repo", "/root/.axon_site/_ro/trn_rl_repo"):
    if _p not in sys.path:
        sys.path.insert(0, _p)

from contextlib import ExitStack
import numpy as np

import concourse.bass as bass
import concourse.tile as tile
from concourse import bacc, mybir
from concourse.masks import make_identity

F32 = mybir.dt.float32
F32R = mybir.dt.float32r
BF16 = mybir.dt.bfloat16
AF = mybir.ActivationFunctionType
ALU = mybir.AluOpType

GH, GW, E, NB, BS, HID, DEPTH = 20, 20, 768, 8, 96, 3072, 12
WC = GW // 2 + 1          # 11
NF = GH * WC              # 220
NPOS = GH * GW            # 400
NIMG = 2                  # images per core
T = NIMG * NPOS           # 800
ET = E // 128             # 6
HT = HID // 128           # 24
OUTP = 4480               # padded head out (35*128)
MT = OUTP // 128          # 35
EPS_LN = 1e-6
EPS_SHR = 1e-8


def build(n_blocks=DEPTH, loop_k=0, with_head=True, dump=()):
    """loop_k>0 wraps the whole forward in a hardware loop (timing variant)."""
    nc = bacc.Bacc(None, target_bir_lowering=False, debug=False)

    # ---- DRAM parameters (per-core shard) ----
    d_xp = nc.dram_tensor("xp", [64, T], F32R, kind="ExternalInput")
    d_pwT = nc.dram_tensor("pwT", [64, E], F32R, kind="ExternalInput")
    d_pb = nc.dram_tensor("pb", [128, ET], F32, kind="ExternalInput")
    d_pos = nc.dram_tensor("pos", [ET, 128, NPOS], F32, kind="ExternalInput")
    d_blkvec = nc.dram_tensor("blkvec", [DEPTH, 128, 54], F32, kind="ExternalInput")
    d_afb = nc.dram_tensor("afb", [DEPTH, BS, 32], F32, kind="ExternalInput")
    d_w1r = nc.dram_tensor("w1r", [DEPTH, BS, NB, BS], BF16, kind="ExternalInput")
    d_w1i = nc.dram_tensor("w1i", [DEPTH, BS, NB, BS], BF16, kind="ExternalInput")
    d_w1in = nc.dram_tensor("w1in", [DEPTH, BS, NB, BS], BF16, kind="ExternalInput")
    d_w2r = nc.dram_tensor("w2r", [DEPTH, BS, NB, BS], BF16, kind="ExternalInput")
    d_w2i = nc.dram_tensor("w2i", [DEPTH, BS, NB, BS], BF16, kind="ExternalInput")
    d_w2in = nc.dram_tensor("w2in", [DEPTH, BS, NB, BS], BF16, kind="ExternalInput")
    d_fc1wT = nc.dram_tensor("fc1wT", [DEPTH, E, HID], BF16, kind="ExternalInput")
    d_fc2wT = nc.dram_tensor("fc2wT", [DEPTH, HID, E], BF16, kind="ExternalInput")
    d_F2 = nc.dram_tensor("F2", [NPOS, 2 * NF], BF16, kind="ExternalInput")
    d_iF2 = nc.dram_tensor("iF2", [2 * NF, NPOS], BF16, kind="ExternalInput")
    d_thr = nc.dram_tensor("thr", [BS, 2 * NF], BF16, kind="ExternalInput")
    d_hwT = nc.dram_tensor("hwT", [E, OUTP], BF16, kind="ExternalInput")
    d_hb = nc.dram_tensor("hb", [128, MT], F32, kind="ExternalInput")
    d_nrm = nc.dram_tensor("nrm", [128, 12], F32, kind="ExternalInput")
    d_y = nc.dram_tensor("y", [OUTP, T], F32, kind="ExternalOutput")
    d_dumps = {}
    for nm in dump:
        d_dumps[nm] = nc.dram_tensor("dump_" + nm, [128, ET * T], F32,
                                     kind="ExternalOutput")

    with ExitStack() as ctx:
        tc = ctx.enter_context(tile.TileContext(nc))
        # ---- pools ----
        const = ctx.enter_context(tc.tile_pool(name="const", bufs=1))
        lnp = ctx.enter_context(tc.tile_pool(name="lnp", bufs=1))     # LN stats tiles
        tmp = ctx.enter_context(tc.tile_pool(name="tmp", bufs=2))     # [128,800] f32 temps
        xsqp = ctx.enter_context(tc.tile_pool(name="xsqp", bufs=2))
        big = ctx.enter_context(tc.tile_pool(name="big", bufs=1))     # x, h, xT, zT, h1
        cbp = ctx.enter_context(tc.tile_pool(name="cbp", bufs=4))     # per-cb freq tiles
        afw = ctx.enter_context(tc.tile_pool(name="afw", bufs=1))     # afno weights
        vecp = ctx.enter_context(tc.tile_pool(name="vecp", bufs=2))   # per-block vectors
        w1p = ctx.enter_context(tc.tile_pool(name="w1p", bufs=6))     # fc1 k-tiles
        w2p = ctx.enter_context(tc.tile_pool(name="w2p", bufs=3))     # fc2 k-tiles
        hwp = ctx.enter_context(tc.tile_pool(name="hwp", bufs=6))     # head k-tiles
        ps_mm = ctx.enter_context(tc.tile_pool(name="ps_mm", bufs=3, space="PSUM"))
        ps_ln = ctx.enter_context(tc.tile_pool(name="ps_ln", bufs=2, space="PSUM"))
        ps_f = ctx.enter_context(tc.tile_pool(name="ps_f", bufs=2, space="PSUM"))
        ps_tp = ctx.enter_context(tc.tile_pool(name="ps_tp", bufs=1, space="PSUM"))

        # ---- constants ----
        ident = const.tile([128, 128], BF16)
        make_identity(nc, ident[:])
        ones_f = const.tile([128, 128], F32)
        nc.vector.memset(ones_f[:], 1.0)
        ones = const.tile([128, 128], F32R)
        nc.scalar.copy(ones[:], ones_f[:])
        epsln = const.tile([128, 1], F32)
        nc.vector.memset(epsln[:], EPS_LN)
        epssh = const.tile([128, 1], F32)
        nc.vector.memset(epssh[:], EPS_SHR)
        zeros440 = const.tile([BS, 2 * NF], F32)
        nc.vector.memset(zeros440[:], 0.0)

        F2sb = const.tile([128, 4 * 2 * NF], BF16)   # kt-major [128|128|128|16, 440]
        for k in range(4):
            p = min(128, NPOS - k * 128)
            nc.sync.dma_start(F2sb[:p, k * 440:(k + 1) * 440],
                              d_F2[k * 128:k * 128 + p, :])
        iF2sb = const.tile([128, 4 * NPOS], BF16)    # kt rows 128|92|128|92
        ifr = [(0, 128), (128, 92), (220, 128), (348, 92)]
        for k, (r0, p) in enumerate(ifr):
            nc.sync.dma_start(iF2sb[:p, k * NPOS:(k + 1) * NPOS],
                              d_iF2[r0:r0 + p, :])
        thr = const.tile([BS, 2 * NF], BF16)
        nc.sync.dma_start(thr[:], d_thr[:])
        pwT = const.tile([64, E], F32R)
        nc.sync.dma_start(pwT[:], d_pwT[:])
        pb = const.tile([128, ET], F32)
        nc.sync.dma_start(pb[:], d_pb[:])
        nrm = const.tile([128, 12], F32)
        nc.sync.dma_start(nrm[:], d_nrm[:])
        hbt = const.tile([128, MT], F32)
        nc.sync.dma_start(hbt[:], d_hb[:])
        xp = const.tile([64, T], F32R)
        nc.sync.dma_start(xp[:], d_xp[:])

        # ---- persistent big tiles ----
        x = big.tile([128, ET * T], F32R, tag="x")         # residual stream
        h = big.tile([128, ET * T], BF16, tag="h")         # LN output
        xT = big.tile([128, NIMG * 4 * E], BF16, tag="xT")  # pos-major h
        zT = big.tile([128, NIMG * 4 * E], BF16, tag="zT")  # freq-major z
        h1 = big.tile([128, HT * NPOS], BF16, tag="h1")    # fc1 out (one n-chunk)

        def layernorm_chunk(src, wcol, bcol, vec, dst, c):
            """One image-chunk of LN: src/dst [128, ET*T], chunk c."""
            psum_s = ps_ln.tile([128, 400], F32, tag="ln", name="psum_s")
            psum_q = ps_ln.tile([128, 400], F32, tag="ln", name="psum_q")
            meanb = lnp.tile([128, 400], F32R, tag="meanb", bufs=2)
            qs = lnp.tile([128, 400], F32, tag="qs", bufs=2)
            rstd = lnp.tile([128, 400], F32, tag="rstd", bufs=2)
            if True:
                for et in range(ET):
                    so = et * T + c * 400
                    xs = xsqp.tile([128, 400], F32R, tag="xsq")
                    if (et + c) % 2 == 0:
                        nc.vector.tensor_mul(xs[:], src[:, so:so + 400],
                                             src[:, so:so + 400])
                    else:
                        nc.scalar.activation(xs[:], src[:, so:so + 400], AF.Square)
                    nc.tensor.matmul(psum_s[:], ones[:], src[:, so:so + 400],
                                     start=(et == 0), stop=(et == ET - 1))
                    nc.tensor.matmul(psum_q[:], ones[:], xs[:],
                                     start=(et == 0), stop=(et == ET - 1))
                nc.scalar.activation(meanb[:], psum_s[:], AF.Identity,
                                     scale=1.0 / E)
                nc.scalar.activation(qs[:], psum_s[:], AF.Square,
                                     scale=1.0 / E)   # mean^2
                nc.vector.scalar_tensor_tensor(
                    out=qs[:], in0=psum_q[:], scalar=1.0 / E,
                    in1=qs[:], op0=ALU.mult, op1=ALU.subtract)
                nc.scalar.activation(rstd[:], qs[:],
                                     AF.Abs_reciprocal_sqrt,
                                     bias=epsln[:], scale=1.0)
                for et in range(ET):
                    so = et * T + c * 400
                    xm = tmp.tile([128, 400], F32, tag="tmp")
                    eng = nc.vector if et % 2 == 0 else nc.gpsimd
                    eng.tensor_sub(xm[:], src[:, so:so + 400], meanb[:])
                    eng.tensor_mul(xm[:], xm[:], rstd[:])
                    nc.vector.tensor_scalar(
                        out=dst[:, so:so + 400], in0=xm[:],
                        scalar1=vec[:, wcol + et: wcol + et + 1],
                        scalar2=vec[:, bcol + et: bcol + et + 1],
                        op0=ALU.mult, op1=ALU.add)

        def body(it=None):
            # ================= patch embed =================
            for et in range(ET):
                xpre = xsqp.tile([128, T], F32, tag="xsq")
                for c in range(2):
                    pp = ps_mm.tile([128, 400], F32, tag="mm")
                    nc.tensor.matmul(pp[:], pwT[:, et * 128:(et + 1) * 128],
                                     xp[:, c * 400:(c + 1) * 400],
                                     start=True, stop=True)
                    nc.scalar.activation(xpre[:, c * 400:(c + 1) * 400], pp[:],
                                         AF.Identity,
                                         bias=pb[:, et:et + 1], scale=1.0)
                post = tmp.tile([128, NPOS], F32, tag="pos")
                nc.sync.dma_start(post[:], d_pos[et])
                for v in range(NIMG):
                    nc.vector.tensor_add(
                        x[:, et * T + v * 400: et * T + (v + 1) * 400],
                        xpre[:, v * 400:(v + 1) * 400], post[:])

            # ================= blocks =================
            for blk in range(n_blocks):
                bv = vecp.tile([128, 54], F32, tag="bv")
                nc.sync.dma_start(bv[:], d_blkvec[blk])
                ab = vecp.tile([BS, 32], F32, tag="ab")
                nc.sync.dma_start(ab[:], d_afb[blk])
                wm = {}
                for nm, dd in (("w1r", d_w1r), ("w1i", d_w1i), ("w1in", d_w1in),
                               ("w2r", d_w2r), ("w2i", d_w2i), ("w2in", d_w2in)):
                    t_ = afw.tile([BS, NB * BS], BF16, tag=nm)
                    nc.sync.dma_start(t_[:], dd[blk])
                    wm[nm] = t_

                # ---- per-image pipeline: LN1 -> tp -> AFNO -> iFFT -> LN2 -> MLP
                chunks = [(0, 128), (128, 128), (256, 128), (384, 16)]
                w1t = [w1p.tile([128, HID], BF16, tag="w1k", name="w1t")
                       for _ in range(ET)]
                for k in range(ET):
                    nc.sync.dma_start(w1t[k][:], d_fc1wT[blk, k * 128:(k + 1) * 128, :])
                for v in range(NIMG):
                    layernorm_chunk(x, 0, 6, bv, h, v)
                    # transposes h(chunk v) -> xT
                    for k, (c0, cn) in enumerate(chunks):
                        for eg, ne in ((0, 4), (4, 2)):
                            pt = ps_tp.tile([128, 512], BF16, tag="tp")
                            for ei in range(ne):
                                et = eg + ei
                                nc.tensor.transpose(
                                    pt[:cn, ei * 128:(ei + 1) * 128],
                                    h[:, et * T + v * 400 + c0:
                                         et * T + v * 400 + c0 + cn],
                                    ident[:])
                            nc.vector.tensor_copy(
                                xT[:cn, (v * 4 + k) * E + eg * 128:
                                        (v * 4 + k) * E + (eg + ne) * 128],
                                pt[:cn, :ne * 128])
                    # AFNO per channel-block (image v)
                    for cb in range(NB):
                        cbs = slice(cb * BS, (cb + 1) * BS)
                        xri = cbp.tile([BS, 2 * NF], BF16, tag="xri")
                        o1r = cbp.tile([BS, NF], BF16, tag="o1r")
                        o1i = cbp.tile([BS, NF], BF16, tag="o1i")
                        o2r = cbp.tile([BS, NF], BF16, tag="o2r")
                        o2i = cbp.tile([BS, NF], BF16, tag="o2i")
                        zr = cbp.tile([BS, NF], BF16, tag="zr")
                        zi = cbp.tile([BS, NF], BF16, tag="zi")
                        pf = ps_f.tile([BS, 2 * NF], F32, tag="psf")
                        for k in range(4):
                            p = min(128, NPOS - k * 128)
                            nc.tensor.matmul(
                                pf[:], xT[:p, (v * 4 + k) * E + cb * BS:
                                          (v * 4 + k) * E + (cb + 1) * BS],
                                F2sb[:p, k * 440:(k + 1) * 440],
                                start=(k == 0), stop=(k == 3))
                        nc.scalar.copy(xri[:], pf[:])
                        xr_v = xri[:, 0:NF]
                        xi_v = xri[:, NF:2 * NF]
                        pm1 = ps_mm.tile([BS, 2 * NF], F32, tag="mm", name="pmix")
                        nc.tensor.matmul(pm1[:, :NF], wm["w1r"][:, cbs], xr_v, start=True, stop=False)
                        nc.tensor.matmul(pm1[:, :NF], wm["w1in"][:, cbs], xi_v, start=False, stop=True)
                        nc.tensor.matmul(pm1[:, NF:], wm["w1i"][:, cbs], xr_v, start=True, stop=False)
                        nc.tensor.matmul(pm1[:, NF:], wm["w1r"][:, cbs], xi_v, start=False, stop=True)
                        nc.scalar.activation(o1r[:], pm1[:, :NF], AF.Relu,
                                             bias=ab[:, cb:cb + 1], scale=1.0)
                        nc.scalar.activation(o1i[:], pm1[:, NF:], AF.Relu,
                                             bias=ab[:, 8 + cb:8 + cb + 1], scale=1.0)
                        pm2 = ps_mm.tile([BS, 2 * NF], F32, tag="mm", name="pmix")
                        nc.tensor.matmul(pm2[:, :NF], wm["w2r"][:, cbs], o1r[:], start=True, stop=False)
                        nc.tensor.matmul(pm2[:, :NF], wm["w2in"][:, cbs], o1i[:], start=False, stop=True)
                        nc.tensor.matmul(pm2[:, NF:], wm["w2i"][:, cbs], o1r[:], start=True, stop=False)
                        nc.tensor.matmul(pm2[:, NF:], wm["w2r"][:, cbs], o1i[:], start=False, stop=True)
                        nc.vector.tensor_scalar(
                            out=o2r[:], in0=pm2[:, :NF], scalar1=ab[:, 16 + cb:16 + cb + 1],
                            scalar2=None, op0=ALU.add)
                        nc.vector.tensor_scalar(
                            out=o2i[:], in0=pm2[:, NF:], scalar1=ab[:, 24 + cb:24 + cb + 1],
                            scalar2=None, op0=ALU.add)
                        s = cbp.tile([BS, NF], BF16, tag="s")
                        s2 = cbp.tile([BS, NF], BF16, tag="s2")
                        nc.vector.tensor_mul(s[:], o2r[:], o2r[:])
                        nc.vector.tensor_mul(s2[:], o2i[:], o2i[:])
                        nc.vector.tensor_add(s[:], s[:], s2[:])
                        inv = cbp.tile([BS, NF], BF16, tag="inv")
                        nc.scalar.activation(inv[:], s[:], AF.Abs_reciprocal_sqrt,
                                             bias=epssh[:BS], scale=1.0)
                        nc.vector.tensor_mul(inv[:], inv[:], thr[:, :NF])
                        shk = cbp.tile([BS, NF], BF16, tag="s")
                        nc.scalar.activation(shk[:], inv[:], AF.Relu,
                                             bias=1.0, scale=-1.0)
                        nc.gpsimd.tensor_mul(zr[:], o2r[:], shk[:])
                        nc.gpsimd.tensor_mul(zi[:], o2i[:], shk[:])
                        # transpose z -> zT, batched psum [re128|im128|re92|im92]
                        pt = ps_tp.tile([128, 512], BF16, tag="tp")
                        nc.tensor.transpose(pt[:128, 0:BS], zr[:, 0:128],
                                            ident[:BS, :BS])
                        nc.tensor.transpose(pt[:128, BS:2 * BS], zi[:, 0:128],
                                            ident[:BS, :BS])
                        nc.tensor.transpose(pt[:92, 2 * BS:3 * BS], zr[:, 128:220],
                                            ident[:BS, :BS])
                        nc.tensor.transpose(pt[:92, 3 * BS:4 * BS], zi[:, 128:220],
                                            ident[:BS, :BS])
                        dst = zT[:128, (v * 4) * E:(v * 4 + 4) * E].rearrange(
                            "p (a b e) -> p a b e", a=2, b=2)[:, :, 0,
                            cb * BS:(cb + 1) * BS]
                        nc.vector.tensor_copy(
                            dst, pt[:128, 0:2 * BS].rearrange(
                                "p (a e) -> p a e", a=2))
                        dst2 = zT[:92, (v * 4) * E:(v * 4 + 4) * E].rearrange(
                            "p (a b e) -> p a b e", a=2, b=2)[:, :, 1,
                            cb * BS:(cb + 1) * BS]
                        nc.vector.tensor_copy(
                            dst2, pt[:92, 2 * BS:4 * BS].rearrange(
                                "p (a e) -> p a e", a=2))
                    # iFFT + residual x = x + h + z (image v)
                    for et in range(ET):
                        so = et * T + v * 400
                        hx = tmp.tile([128, 400], F32, tag="tmp")
                        nc.gpsimd.tensor_add(hx[:], x[:, so:so + 400],
                                             h[:, so:so + 400])
                        pz = ps_mm.tile([128, 400], F32, tag="mm")
                        for kt, (_, p) in enumerate(ifr):
                            nc.tensor.matmul(
                                pz[:], zT[:p, (v * 4 + kt) * E + et * 128:
                                          (v * 4 + kt) * E + (et + 1) * 128],
                                iF2sb[:p, kt * NPOS:(kt + 1) * NPOS],
                                start=(kt == 0), stop=(kt == 3))
                        nc.vector.tensor_add(x[:, so:so + 400], hx[:], pz[:])
                    # LN2 chunk v
                    layernorm_chunk(x, 12, 18, bv, h, v)
                    # MLP (n = v)
                    n = v
                    for m in range(HT):
                        pp = ps_mm.tile([128, 400], F32, tag="mm")
                        for k in range(ET):
                            nc.tensor.matmul(
                                pp[:], w1t[k][:, m * 128:(m + 1) * 128],
                                h[:, k * T + n * 400: k * T + n * 400 + 400],
                                start=(k == 0), stop=(k == ET - 1))
                        nc.scalar.activation(h1[:, m * 400:(m + 1) * 400], pp[:],
                                             AF.Gelu,
                                             bias=bv[:, 24 + m:24 + m + 1], scale=1.0)
                    pouts = [ps_mm.tile([128, 400], F32, tag="mm", name="pfc2") for _ in range(3)] + \
                            [ps_f.tile([128, 400], F32, tag="psf", name="pfc2b") for _ in range(2)] + \
                            [ps_ln.tile([128, 400], F32, tag="ln", name="pfc2c")]
                    for k in range(HT):
                        w2t = w2p.tile([128, E], BF16, tag="w2k")
                        nc.sync.dma_start(w2t[:], d_fc2wT[blk, k * 128:(k + 1) * 128, :])
                        for m in range(ET):
                            nc.tensor.matmul(
                                pouts[m][:], w2t[:, m * 128:(m + 1) * 128],
                                h1[:, k * 400:(k + 1) * 400],
                                start=(k == 0), stop=(k == HT - 1))
                    for m in range(ET):
                        nc.vector.scalar_tensor_tensor(
                            out=x[:, m * T + n * 400: m * T + n * 400 + 400],
                            in0=pouts[m][:], scalar=bv[:, 48 + m:48 + m + 1],
                            in1=x[:, m * T + n * 400: m * T + n * 400 + 400],
                            op0=ALU.add, op1=ALU.add)

            if "x" in d_dumps:
                nc.sync.dma_start(d_dumps["x"][:], x[:].bitcast(F32))
            if "h" in d_dumps:
                ht_ = tmp.tile([128, T], F32, tag="tmp")
                for et in range(ET):
                    nc.vector.tensor_copy(ht_[:], h[:, et * T:(et + 1) * T])
                    nc.sync.dma_start(d_dumps["h"][:, et * T:(et + 1) * T], ht_[:])

            # ================= head =================
            if with_head:
                for c_ in range(2):
                    layernorm_chunk(x, 0, 6, nrm, h, c_)
                # m-groups of 6 tiles; per group load 6 k-slices [128, <=768]
                for mg in range(0, MT, 3):
                    nmg = min(3, MT - mg)
                    mw = nmg * 128
                    hk = [hwp.tile([128, 384], BF16, tag="hw", name="hk") for _ in range(ET)]
                    for k in range(ET):
                        nc.sync.dma_start(
                            hk[k][:, :mw],
                            d_hwT[k * 128:(k + 1) * 128, mg * 128:mg * 128 + mw])
                    for n in range(2):
                        pouts = [ps_mm.tile([128, 400], F32, tag="mm", name="phd")
                                 for _ in range(min(nmg, 4))] + \
                                [ps_f.tile([128, 400], F32, tag="psf", name="phdb")
                                 for _ in range(max(0, nmg - 4))]
                        for k in range(ET):
                            for mi in range(nmg):
                                nc.tensor.matmul(
                                    pouts[mi][:], hk[k][:, mi * 128:(mi + 1) * 128],
                                    h[:, k * T + n * 400: k * T + n * 400 + 400],
                                    start=(k == 0), stop=(k == ET - 1))
                        for mi in range(nmg):
                            m = mg + mi
                            yo = tmp.tile([128, 400], F32, tag="yo")
                            nc.scalar.activation(yo[:], pouts[mi][:], AF.Identity,
                                                 bias=hbt[:, m:m + 1], scale=1.0)
                            nc.sync.dma_start(
                                d_y[m * 128:(m + 1) * 128, n * 400:(n + 1) * 400],
                                yo[:])

        if loop_k and loop_k > 1:
            with tc.For_i(0, loop_k, 1):
                body()
        else:
            body()

    nc.compile()
    return nc


# ---------------- host-side prep ----------------
def build_dft_mats():
    h = np.arange(GH); w = np.arange(GW)
    hf = np.arange(GH); wf = np.arange(WC)
    pos_h = np.repeat(h, GW); pos_w = np.tile(w, GH)
    f_h = np.repeat(hf, WC); f_w = np.tile(wf, GH)
    ang = 2 * np.pi * (np.outer(pos_h, f_h) / GH + np.outer(pos_w, f_w) / GW)
    scale = 1.0 / np.sqrt(NPOS)
    F2 = np.concatenate([np.cos(ang) * scale, -np.sin(ang) * scale], axis=1)
    wgt = np.where((f_w == 0) | (f_w == WC - 1), 1.0, 2.0)
    AR = (np.cos(ang) * wgt * scale).T
    AI = (-np.sin(ang) * wgt * scale).T
    iF2 = np.concatenate([AR, AI], axis=0)
    return F2.astype(np.float32), iF2.astype(np.float32)


def build_threshold(base=0.01, kdec=20.0, mn=0.0005):
    fu = np.fft.fftfreq(GH)[:, None]
    fv = np.fft.rfftfreq(GW)[None, :]
    k = np.sqrt(fu ** 2 + fv ** 2) * max(GH, GW)
    t = np.maximum(base * np.exp(-0.5 * (k / kdec) ** 2), mn)
    return t.astype(np.float32).ravel()  # (220,) hf-major


def host_prep(inputs, n_cores=8):
    """Returns (shared_map, per_core_maps). shared entries replicated."""
    import ml_dtypes
    bf = ml_dtypes.bfloat16
    f32 = np.float32
    B = 16
    F2, iF2 = build_dft_mats()
    thr220 = build_threshold()

    def colpack(v, ncol):  # (ncol*128,) -> [128, ncol]
        return np.ascontiguousarray(v.reshape(ncol, 128).T)

    sh = {}
    sh["pwT"] = np.ascontiguousarray(
        inputs["patch_w"].reshape(E, 64).T).astype(f32)
    sh["pb"] = colpack(inputs["patch_b"], ET)
    sh["pos"] = np.ascontiguousarray(
        inputs["pos_embed"].reshape(NPOS, E).T.reshape(ET, 128, NPOS)).astype(f32)
    bv = np.zeros((DEPTH, 128, 54), f32)
    for d in range(DEPTH):
        bv[d, :, 0:6] = colpack(inputs["n1w"][d], ET)
        bv[d, :, 6:12] = colpack(inputs["n1b"][d], ET)
        bv[d, :, 12:18] = colpack(inputs["n2w"][d], ET)
        bv[d, :, 18:24] = colpack(inputs["n2b"][d], ET)
        bv[d, :, 24:48] = colpack(inputs["fc1b"][d], HT)
        bv[d, :, 48:54] = colpack(inputs["fc2b"][d], ET)
    sh["blkvec"] = bv
    ab = np.zeros((DEPTH, BS, 32), f32)
    for d in range(DEPTH):
        ab[d, :, 0:8] = inputs["fb1"][d, 0].T
        ab[d, :, 8:16] = inputs["fb1"][d, 1].T
        ab[d, :, 16:24] = inputs["fb2"][d, 0].T
        ab[d, :, 24:32] = inputs["fb2"][d, 1].T
    sh["afb"] = ab
    # afno weights: [d, cb, i, o] -> [d, i, cb, o]
    tr = lambda a: np.ascontiguousarray(a.transpose(0, 2, 1, 3)).astype(bf)
    sh["w1r"] = tr(inputs["fw1"][:, 0])
    sh["w1i"] = tr(inputs["fw1"][:, 1])
    sh["w1in"] = tr(-inputs["fw1"][:, 1])
    sh["w2r"] = tr(inputs["fw2"][:, 0])
    sh["w2i"] = tr(inputs["fw2"][:, 1])
    sh["w2in"] = tr(-inputs["fw2"][:, 1])
    sh["fc1wT"] = np.ascontiguousarray(
        inputs["fc1w"].transpose(0, 2, 1)).astype(bf)
    sh["fc2wT"] = np.ascontiguousarray(
        inputs["fc2w"].transpose(0, 2, 1)).astype(bf)
    sh["F2"] = F2.astype(bf)
    sh["iF2"] = iF2.astype(bf)
    sh["thr"] = np.tile(np.concatenate([thr220, thr220])[None, :], (BS, 1)).astype(bf)
    hwT = np.zeros((E, OUTP), f32)
    hwT[:, :inputs["headw"].shape[0]] = inputs["headw"].T
    sh["hwT"] = hwT.astype(bf)
    hb = np.zeros(OUTP, f32)
    hb[:inputs["headb"].shape[0]] = inputs["headb"]
    sh["hb"] = colpack(hb, MT)
    nrmv = np.zeros((128, 12), f32)
    nrmv[:, 0:6] = colpack(inputs["normw"], ET)
    nrmv[:, 6:12] = colpack(inputs["normb"], ET)
    sh["nrm"] = nrmv

    # per-core x patches
    x = inputs["x"]
    t = x.reshape(B, 1, GH, 8, GW, 8).transpose(0, 2, 4, 1, 3, 5).reshape(B, NPOS, 64)
    per_core = []
    for c in range(n_cores):
        xp = np.concatenate([t[2 * c].T, t[2 * c + 1].T], axis=1)  # [64, 800]
        per_core.append({"xp": np.ascontiguousarray(xp).astype(f32)})
    in_maps = [{**sh, **pc} for pc in per_core]
    return in_maps


def host_post(results):
    """results: list of 8 per-core dicts with 'y' [OUTP, 800] -> (16,69,160,160)"""
    B, OC, P = 16, 69, 8
    ys = []
    for c in range(8):
        y = results[c]["y"][:4416]          # [4416, 800]
        for v in range(NIMG):
            ys.append(y[:, v * 400:(v + 1) * 400])
    Y = np.stack(ys)                         # (16, 4416, 400)
    Y = Y.reshape(B, P, P, OC, GH, GW)
    Y = Y.transpose(0, 3, 4, 1, 5, 2)
    return np.ascontiguousarray(Y.reshape(B, OC, 160, 160))


# ======================= harness entry point =======================
_NC_CACHE = {}


def _get_nc():
    if "nc" not in _NC_CACHE:
        _NC_CACHE["nc"] = build(n_blocks=DEPTH, loop_k=0, with_head=True)
    return _NC_CACHE["nc"]


def kernel(**inputs):
    """AFNONet forward on 8 TRN2 NeuronCores (data-parallel over batch).

    Takes the full unsharded inputs (as produced by setup_inputs()),
    returns the full (16, 69, 160, 160) float32 output.
    """
    from concourse.bass_utils import run_bass_kernel_spmd

    inputs = {k: np.asarray(v) for k, v in inputs.items()}
    in_maps = host_prep(inputs, n_cores=8)
    nc = _get_nc()
    res = run_bass_kernel_spmd(nc, in_maps, core_ids=list(range(8)))
    return host_post(res.results).astype(np.float32)

